# revision 26
# baseline (speedup 1.0000x reference)
"""Block self-attention (Gaussian kernel weights) Trainium2 Bass kernel, v9.

For each independent block of B=1024 rows of `features` [262144, 128]:
    w = exp(-(sq_i + sq_j - 2 x@x^T)/25.6);  out = (w @ x)/B
Blocks are data-parallel across 8 NeuronCores (32 blocks per core).

Key algebra: w = D_e A D_e with A = exp(2G/25.6) symmetric, e = exp(-sq/25.6).
  out_j = (e_j/B) * sum_i A_ij * (e_i x_i)
The diagonal i=j term equals x_j/B exactly (exponents cancel in fp32).

Measured HW facts driving v7 (v5 critical-cycle analysis):
 - ALL DMA-transposes + DMAs serialize into one global chain (deadlock
   avoidance): v5's cycle = tout(1.26) + mir(5.7) + xT(1.9) + store issue
   (0.6) + wait-store-complete(2.5) ~= 13.4us/block = the period.
 - The xbar mirror transpose is descriptor-rate-bound (~19ns per 256B
   column over 16 engines): 4736 cols -> 5.7us serial.
 - PE matmul stream is 0.71ns/col (~8us/block busy) and a [128,128] PE
   transpose costs ~90ns -> the 28-chunk mirror fits in PE+DVE slack.
v9 therefore computes every mirror chunk on the PE (transpose via
identity into a 1-bank PSUM staging tile) + one DVE copy per source row
into the same AM layout, leaving only xT + outT + store on the DMA
chain.  The mirror stage runs one pipeline iteration AFTER the block's
ACT (stage k-3): PE-transposing a row right after ACT wrote it was racy
(~25% inf runs; LDWEIGHTS prefetch vs cross-engine semaphore).  Keeping
the xT transpose on the DMA xbar also proved necessary: a PE-transposed
xT (v11) raced too (nan/hang), likely from PSUM staging-ring churn.
PSUM: G tiles [128,1024] x2 bufs (4) + acc 2 + staging 2 = 8 banks.

Layout (unchanged from v5): p-major row labeling (row = b*1024 + 8p + c)
for 4KB-contiguous HBM DMA; compact A6 [128, 5120] with row pitches
1024,1024,768,768,512,512,256,256; AM [128, 37, 128] mirror chunks.
"""

import math
import os

os.environ.setdefault("NEURON_RT_RESET_CORES", "1")

import numpy as np

import concourse.bass as bass
import concourse.tile as tile
from concourse import bacc, mybir
from concourse.bass_utils import run_bass_kernel_spmd
from concourse.masks import make_identity

N_TOTAL = 262144
D = 128
B = 1024
NCORES = 8
ROWS_PER_CORE = N_TOTAL // NCORES   # 32768
NB_FULL = ROWS_PER_CORE // B        # 32 blocks per core
C = B // 128                        # 8 row-chunks per block

F32 = mybir.dt.float32
BF16 = mybir.dt.bfloat16
FP16 = mybir.dt.float16

SIGMA2X2 = 2.0 * (D / 10.0)         # 25.6
G_SCALE = 2.0 / SIGMA2X2            # 0.078125
NEG_INV = -1.0 / SIGMA2X2           # -0.0390625
# outT is cast fp32->fp16 with a 1/OSC scale to keep away from fp16 max;
# the tail multiplies by e_j*OSC/B.
OSC = 16.0

EXP = mybir.ActivationFunctionType.Exp
MULT = mybir.AluOpType.mult

# trapezoid packing in PSUM: row c covers cols [128c, 1024) => width 1024-128c
ROW_W = [B - 128 * c for c in range(C)]
CUM = [0]
for w in ROW_W:
    CUM.append(CUM[-1] + w)
PACK = CUM[C]                        # 4608
TILE_W = 1024
NT = (PACK + TILE_W - 1) // TILE_W   # 5 G-psum tiles (last 512 wide)

# Compact A6 row-layout: row c stored for j in [J0[c], 1024)
J0 = [0, 0, 256, 256, 512, 512, 768, 768]
PITCH = [1024 - j for j in J0]       # 1024,1024,768,768,512,512,256,256
BASE = [0]
for pw in PITCH:
    BASE.append(BASE[-1] + pw)
A6_W = BASE[C]                       # 5120
MIR_LO = 128
MIR_HI = BASE[6] + PITCH[6]          # 4864 (end of row 6)
NK = (MIR_HI - MIR_LO) // 128        # 37 chunks in AM


def a6_off(c, j):
    """Offset of A[row c, col j] in the compact A6 tile."""
    assert j >= J0[c]
    return BASE[c] + (j - J0[c])


def am_idx(t, s):
    """AM chunk index holding mirror chunk A[i in t, j in s] (t > s).
    For fixed s, consecutive t are consecutive chunks."""
    col = BASE[s] + 128 * t - J0[s]
    assert MIR_LO <= col < MIR_HI
    return col // 128 - 1


def mm1_pieces():
    """(tile_idx, off_in_tile, row_c, col_start, n) split at 512 banks."""
    ps = []
    for c in range(C):
        s = CUM[c]
        while s < CUM[c + 1]:
            e = min(CUM[c + 1], (s // 512 + 1) * 512)
            ps.append((s // TILE_W, s % TILE_W, c, 128 * c + (s - CUM[c]), e - s))
            s = e
    return ps


MM1_PIECES = mm1_pieces()
MM1_BY_TILE = [[p for p in MM1_PIECES if p[0] == t] for t in range(NT)]


def act_pieces():
    """(tile_idx, off_in_tile, row_c, col_start, n, row_done) split only at
    tile bounds; row_done flags the piece that finishes row c."""
    ps = []
    for c in range(C):
        s = CUM[c]
        while s < CUM[c + 1]:
            e = min(CUM[c + 1], (s // TILE_W + 1) * TILE_W)
            ps.append((s // TILE_W, s % TILE_W, c, 128 * c + (s - CUM[c]),
                       e - s, e == CUM[c + 1]))
            s = e
    return ps


ACT_PIECES = act_pieces()
ACT_BY_TILE = [[p for p in ACT_PIECES if p[0] == t] for t in range(NT)]


def mm2_half_pieces(h):
    """MM pieces for j in [512h, 512h+512): list of (c, kind, js, je) in
    emission order, with per-piece (start, stop) accumulation flags.
    Mirror pieces are emitted per 128-chunk (PE instr overhead ~ 0)."""
    lo, hi = 512 * h, 512 * h + 512
    out = []
    for c in range(C):
        # mirror: j in [0, 128c); direct: j in [128c, 1024)
        mjs, mje = max(0, lo), min(128 * c, hi)
        for s in range(mjs // 128, max(mjs, mje) // 128):
            out.append((c, "mir", 128 * s, 128 * (s + 1)))
        djs, dje = max(128 * c, lo), min(B, hi)
        if dje > djs:
            out.append((c, "dir", djs, dje))
    flags = [(i == 0, i == len(out) - 1) for i in range(len(out))]
    return list(zip(out, flags))


MM2_HALF = [mm2_half_pieces(0), mm2_half_pieces(1)]


def build(nb: int = NB_FULL) -> bacc.Bacc:
    rows = nb * B
    nc = bacc.Bacc("TRN2", target_bir_lowering=False, debug=False)

    fin = nc.dram_tensor("features", [rows, D], F32, kind="ExternalInput").ap()
    fout = nc.dram_tensor("out", [rows, D], F32, kind="ExternalOutput").ap()

    # p-major row labeling: row index = b*1024 + p*8 + c -> per-partition
    # HBM spans are 8*128*4B = 4KB contiguous
    fin_v = fin.rearrange("(b p c) d -> b p (c d)", p=128, c=C)
    fout_v = fout.rearrange("(b p c) d -> b p c d", p=128, c=C)

    with tile.TileContext(nc) as tc:
        with (
            tc.tile_pool(name="const", bufs=1) as cpool,
            tc.tile_pool(name="xr", bufs=3) as xrpool,
            tc.tile_pool(name="xt", bufs=2) as xtpool,
            tc.tile_pool(name="y", bufs=5) as ypool,
            tc.tile_pool(name="sq", bufs=4) as sqpool,
            tc.tile_pool(name="a6", bufs=3) as a6pool,    # A rows bf16
            tc.tile_pool(name="am", bufs=3) as ampool,    # mirror [128,37,128]
            tc.tile_pool(name="ot", bufs=2) as otpool,    # outT_sb fp16
            tc.tile_pool(name="tr", bufs=2) as trpool,    # trd fp16
            tc.tile_pool(name="of", bufs=2) as ofpool,    # out_final fp32
            tc.tile_pool(name="gp", bufs=2, space="PSUM") as gpool,
            tc.tile_pool(name="acc", bufs=2, space="PSUM") as accpool,
            tc.tile_pool(name="mt", bufs=2, space="PSUM") as mtpool,
        ):
            identb = cpool.tile([128, 128], BF16)
            make_identity(nc, identb[:])

            state: dict[int, dict] = {}

            def load(b):
                xr = xrpool.tile([128, C, D], BF16)
                nc.gpsimd.dma_start(
                    out=xr[:].rearrange("p c d -> p (c d)"), in_=fin_v[b]
                )  # SWDGE cast DMA, 4KB/partition contiguous
                state[b] = dict(xr=xr)

            def prep_mul(b):
                st = state[b]
                xr = st["xr"]
                xsq = sqpool.tile([128, C * D], BF16, tag="xsq")
                nc.gpsimd.tensor_mul(
                    xsq[:], xr[:].rearrange("p c d -> p (c d)"),
                    xr[:].rearrange("p c d -> p (c d)"),
                )
                st["xsq"] = xsq

            def prep(b):
                st = state[b]
                xsq = st.pop("xsq")
                sqcol = sqpool.tile([128, C], F32, tag="sqc")
                nc.vector.tensor_reduce(
                    sqcol[:], xsq[:].rearrange("p (c d) -> p c d", d=D),
                    axis=mybir.AxisListType.X, op=mybir.AluOpType.add,
                )
                bias_col = sqpool.tile([128, C], F32, tag="bia")
                nc.vector.tensor_scalar_mul(bias_col[:], sqcol[:], NEG_INV)
                st["bias_col"] = bias_col

            def escalc(b):
                # escB[p,c,d] = exp(-sq[p,c]/25.6): one ACT instr on a
                # stride-0-broadcast input, fp16 out
                st = state[b]
                escB = ypool.tile([128, C, D], FP16, tag="escB")
                nc.scalar.activation(
                    escB[:],
                    st.pop("bias_col")[:].unsqueeze(2).broadcast_to([128, C, D]),
                    EXP,
                )
                st["escB"] = escB

            def ymul(b):
                st = state[b]
                y = ypool.tile([128, C, D], BF16, tag="y")
                nc.vector.tensor_mul(y[:], st["xr"][:], st["escB"][:])
                st["y"] = y

            def xtrans(b):
                # x^T per 128-chunk via one DMA xbar transpose
                st = state[b]
                xT = xtpool.tile([128, C, 128], BF16)
                nc.sync.dma_start_transpose(
                    out=xT[:], in_=st["xr"][:].rearrange("p c d -> p (c d)")
                )
                st["xT"] = xT

            def m1_tile(b, t):
                st = state[b]
                if t == 0:
                    st["g"] = {}
                    st["a6"] = a6pool.tile([128, A6_W], BF16, name="a6", tag="a6")
                g = gpool.tile([128, min(TILE_W, PACK - t * TILE_W)], F32,
                               tag="g")
                st["g"][t] = g
                xT = st["xT"][:].rearrange("p c d -> p (c d)")
                for (_, off, c, col, n) in MM1_BY_TILE[t]:
                    nc.tensor.matmul(
                        g[:, off:off + n],
                        lhsT=st["xT"][:, c, :],
                        rhs=xT[:, col:col + n],
                        start=True, stop=True,
                    )

            def mir_row(b, s):
                # mirror chunks (t, s) for t > s: PE transposes of A6 row-s
                # chunks into a 1-bank PSUM staging tile, then one DVE copy
                # into AM.  Runs one iteration AFTER the block's ACT so the
                # PE weight-load never races a just-written A6 row.
                st = state[b]
                if s == 0:
                    st["am"] = ampool.tile([128, NK, 128], BF16, name="am",
                                           tag="am")
                nchunks = C - 1 - s
                trt = mtpool.tile([128, nchunks, 128], BF16, tag="mt")
                for i, t in enumerate(range(s + 1, C)):
                    lo = a6_off(s, 128 * t)
                    nc.tensor.transpose(
                        out=trt[:, i, :], in_=st["a6"][:, lo:lo + 128],
                        identity=identb[:],
                    )
                k0 = am_idx(s + 1, s)
                nc.vector.tensor_copy(
                    st["am"][:, k0:k0 + nchunks, :].rearrange(
                        "p k d -> p (k d)"),
                    trt[:].rearrange("p k d -> p (k d)"),
                )

            def act_tile(b, t):
                # exp(G) pieces of psum tile t -> compact A6 row segments
                st = state[b]
                g = st["g"].pop(t)
                for (_, off, c, col, n, done) in ACT_BY_TILE[t]:
                    lo = a6_off(c, col)
                    nc.scalar.activation(
                        st["a6"][:, lo:lo + n], g[:, off:off + n], EXP,
                        scale=G_SCALE,
                    )

            def mm2_half(b, h):
                st = state[b]
                if h == 0:
                    st["ot"] = otpool.tile([128, B], FP16, name="ot", tag="ot")
                o = accpool.tile([128, 512], F32, tag="o")
                st["o%d" % h] = o
                for (c, kind, js, je), (start, stop) in MM2_HALF[h]:
                    if kind == "mir":
                        rhs = st["am"][:, am_idx(c, js // 128), :]
                    else:
                        lo = a6_off(c, js)
                        rhs = st["a6"][:, lo:lo + (je - js)]
                    nc.tensor.matmul(
                        o[:, js - 512 * h:je - 512 * h],
                        lhsT=st["y"][:, c, :],
                        rhs=rhs,
                        start=start, stop=stop,
                    )

            def cast_half(b, h):
                st = state[b]
                nc.vector.tensor_scalar_mul(
                    st["ot"][:, h * 512:(h + 1) * 512],
                    st.pop("o%d" % h)[:], 1.0 / OSC
                )

            def tout(b):
                st = state[b]
                trd = trpool.tile([128, C, 128], FP16)
                nc.sync.dma_start_transpose(out=trd[:], in_=st.pop("ot")[:])
                st["trd"] = trd

            def tail(b):
                # out = (trd * OSC/B) * e_j  (one fused STT)
                st = state[b]
                of = ofpool.tile([128, C, D], F32)
                nc.vector.scalar_tensor_tensor(
                    out=of[:], in0=st["trd"][:], scalar=float(OSC / B),
                    in1=st["escB"][:], op0=MULT, op1=MULT,
                )
                st["of"] = of

            def store(b):
                st = state.pop(b)
                nc.sync.dma_start(out=fout_v[b], in_=st["of"][:])

            # software pipeline: iteration k handles load(k), prep/xtrans(k-1),
            # m1/act(k-2), PE-mirror(k-3), mm2/epilogue(k-4)
            for k in range(nb + 4):
                bl, bp, bm, bq, be = k, k - 1, k - 2, k - 3, k - 4
                if 0 <= bp < nb:
                    prep_mul(bp)   # gpsimd: ahead of load(bl) so its DMASW
                                   # wait covers only the previous load
                if bl < nb:
                    load(bl)
                if 0 <= bp < nb:
                    prep(bp)
                if 0 <= bm < nb:
                    m1_tile(bm, 0)
                    act_tile(bm, 0)
                if 0 <= bq < nb:
                    mir_row(bq, 0)
                if 0 <= be < nb:
                    mm2_half(be, 0)
                if 0 <= bm < nb:
                    m1_tile(bm, 1)
                    act_tile(bm, 1)
                if 0 <= bq < nb:
                    mir_row(bq, 1)
                    mir_row(bq, 2)
                if 0 <= be < nb:
                    cast_half(be, 0)
                if 0 <= bm < nb:
                    m1_tile(bm, 2)
                    act_tile(bm, 2)
                if 0 <= bq < nb:
                    mir_row(bq, 3)
                if 0 <= be < nb:
                    mm2_half(be, 1)
                    cast_half(be, 1)
                    tout(be)
                if 0 <= bm < nb:
                    m1_tile(bm, 3)
                    act_tile(bm, 3)
                if 0 <= bq < nb:
                    mir_row(bq, 4)
                    mir_row(bq, 5)
                if 0 <= bm < nb:
                    m1_tile(bm, 4)
                    act_tile(bm, 4)
                if 0 <= bq < nb:
                    mir_row(bq, 6)
                if 0 <= bp < nb:
                    xtrans(bp)
                    escalc(bp)
                    ymul(bp)
                if 0 <= be < nb:
                    tail(be)
                    store(be)

    nc.compile()
    return nc


_CACHE: dict[int, bacc.Bacc] = {}


def _get_nc(nb: int = NB_FULL) -> bacc.Bacc:
    if nb not in _CACHE:
        _CACHE[nb] = build(nb)
    return _CACHE[nb]


def run(features: np.ndarray, nc: bacc.Bacc | None = None, **spmd_kwargs):
    """Shard rows across 8 cores, run, gather. Returns (out, BassKernelResults)."""
    features = np.ascontiguousarray(features, dtype=np.float32)
    assert features.shape == (N_TOTAL, D)
    if nc is None:
        nc = _get_nc()
    core_ids = list(range(NCORES))
    shards = np.split(features, NCORES, axis=0)
    in_maps = [{"features": s} for s in shards]
    res = run_bass_kernel_spmd(nc, in_maps, core_ids, **spmd_kwargs)
    out = np.concatenate([res.results[i]["out"] for i in range(NCORES)], axis=0)
    return out, res


def kernel(features: np.ndarray) -> np.ndarray:
    out, _ = run(features)
    return out
